# revision 1
# baseline (speedup 1.0000x reference)
"""Trainium2 Bass kernel for CausalSelfAttention (GQA, RoPE, prefill).

Tensor-parallel over the 8 query groups: core g owns query heads
[4g, 4g+4) and kv head g.  Each core computes a partial output
(full-shape, fp16) that the host sums in fp32.

Per-core pipeline (one NeuronCore, Tile-scheduled; every engine stream
is in-order, so EMISSION ORDER is the schedule):
  1. qkvT = wqkvT.T @ xT (fp16 matmuls).  Batch 0 runs flat 6-PSUM
     k-sweeps; batch 1 runs two 3-PSUM m-groups over a resident xt set,
     freeing 4 PSUM banks so batch-0's attention SCORES + exp (ACT) are
     interleaved INTO batch-1's qkv stream — by the time PE reaches the
     PV stage, every batch-0 exp is already in SBUF.
  2. RoPE in fp16 on DVE, v -> token-major via PE transpose spread
     across the n-chunks.
  3. scores KV-MAJOR (scoresT = kT.T @ qT, 6-deep PSUM rotation), exp
     on ACT straight into the PV rhs layout, causal-diagonal mask on
     GpSimd.  Softmax denominator: DVE pre-reduces the kv-chunk blocks
     to a [128,T] partial (deferred for batch 1 so it cannot head-of-
     line block DVE), one ones[128x128] matmul does the cross-partition
     reduce AND the 128-way broadcast, DVE reciprocal, normalization
     fused into the PSUM->SBUF move of y.
  4. out_partial = yT.T @ wprojT; batch-0 proj chunks are emitted
     between batch-1's PV heads as PE filler for the ACT-paced exp.
     Output DMAs split per 1K columns (per 512 for the last chunk, with
     its PSUM->SBUF copies alternating ACT/DVE) to shrink the tail.

TimelineSim: 331.5 us vs a 308 us PE-busy floor (fp16 PE work:
qkv 164 + scores 15.4 + PV 15.4 + denom 3.4 + proj 109 us); the
baseline this session started from simulated at 384.9 us.
"""

import os
import numpy as np

FLAGS = set(os.environ.get('KFLAGS', 'quarter_dma,ymul_dve,tail_split,ilv,acc6,late_part,vt_split,tail_alt,altcopy').split(','))

B, T, NE, NH, NQG, HS = 2, 1024, 4096, 32, 8, 128
QPK = NH // NQG          # 4 query heads per kv group
NT = B * T               # 2048 tokens
GW = (QPK + 2) * HS      # 768 qkv rows per group
GQ = QPK * HS            # 512 q cols per group
P = 128
NCORES = 8
KC = NE // P             # 32 contraction chunks for qkv proj
MC = GW // P             # 6 qkv feature chunks
TC8 = T // P             # 8 token chunks per batch
SCALE = 1.0 / float(np.sqrt(HS))

_CACHE = {}


def _split_waits(nc, mybir, max_waits=1):
    """walrus in this container rejects >1 sync-wait per instruction;
    hoist extras onto single-wait NoOps just before (equivalent since
    semaphores are monotonic and a sequencer executes in order)."""
    for fn in nc.m.functions:
        for blk in fn.blocks:
            new_list, changed = [], False
            for inst in blk.instructions:
                si = getattr(inst, "sync_info", None)
                if si is not None and len(si.on_wait) > max_waits:
                    waits = list(si.on_wait)
                    for i, w in enumerate(waits[:-max_waits]):
                        nop = mybir.InstNoOp(
                            name=f"{inst.name}-wsplit-{i}", ins=[], outs=[],
                            engine=inst.engine)
                        nop.sync_info = mybir.SyncInfo(on_wait=[w], on_update=[])
                        new_list.append(nop)
                    inst.sync_info = mybir.SyncInfo(
                        on_wait=waits[-max_waits:], on_update=list(si.on_update))
                    changed = True
                new_list.append(inst)
            if changed:
                blk.instructions = new_list


def _build_nc(reps=1, split_waits=True):
    import concourse.bass as bass
    import concourse.mybir as mybir
    import concourse.tile as tile
    from contextlib import ExitStack

    f32 = mybir.dt.float32
    f16 = mybir.dt.float16

    nc = bass.Bass()
    xT_d = nc.dram_tensor("xT", [NE, NT], f16, kind="ExternalInput")
    wqkvT_d = nc.dram_tensor("wqkvT", [NE, GW], f16, kind="ExternalInput")
    wprojT_d = nc.dram_tensor("wprojT", [GQ, NE], f16, kind="ExternalInput")
    cc_d = nc.dram_tensor("cc", [P, NT], f16, kind="ExternalInput")
    ss_d = nc.dram_tensor("ss", [P, NT], f16, kind="ExternalInput")
    mask_d = nc.dram_tensor("maskT", [P, P], f16, kind="ExternalInput")
    negtri_d = nc.dram_tensor("negtri", [P, P], f32, kind="ExternalInput")
    ones2d_d = nc.dram_tensor("ones2d", [P, P], f16, kind="ExternalInput")
    ident16_d = nc.dram_tensor("ident16", [P, P], f16, kind="ExternalInput")
    out_d = nc.dram_tensor("out", [NT, NE], f16, kind="ExternalOutput")

    # column offset of kv-chunk c's block inside the expT tile
    offs, acc = [], 0
    for c in range(TC8):
        offs.append(acc)
        acc += (TC8 - c) * P

    with tile.TileContext(nc) as tc:
      for _rep in range(reps):
        sL = ExitStack()   # left-side long-lived pools (y, wp, ob)
        sR = ExitStack()   # right-side pools (qk16, attention-era)
        try:
            # const: left
            const = sL.enter_context(tc.tile_pool(name="const", bufs=1))
            cc = const.tile([P, NT], f16)
            ss = const.tile([P, NT], f16)
            maskT = const.tile([P, P], f16)
            negtri = const.tile([P, P], f32)
            ones2d = const.tile([P, P], f16)
            ident16 = const.tile([P, P], f16)

            def alloc_yw_pools():
                y_pool = sL.enter_context(tc.tile_pool(name="y", bufs=1))
                y_sb = y_pool.tile([P, QPK, NT], f16)
                wp_pool = sL.enter_context(tc.tile_pool(name="wp", bufs=1))
                wp = wp_pool.tile([P, QPK, NE], f16)
                return y_sb, wp
            wpr = wprojT_d[:].rearrange("(kc p) n -> p kc n", p=P)
            if 'early_pools' in FLAGS or 'pv_in_p1' in FLAGS:
                y_sb, wp = alloc_yw_pools()

            # qk16 on the right: lives through attention
            qk16 = sR.enter_context(tc.tile_pool(name="qk16", bufs=1, side="right"))
            q16 = qk16.tile([P, QPK, NT], f16)
            k16 = qk16.tile([P, NT], f16)
            vtm = qk16.tile([P, B * TC8, P], f16)

            ILV = 'ilv' in FLAGS
            att = {}

            def make_att_pools():
                att['expT'] = sR.enter_context(
                    tc.tile_pool(name="expT",
                                 bufs=(4 if ('rope_half1' in FLAGS
                                             or ('ilv' in FLAGS
                                                 and 'acc6' not in FLAGS))
                                       else 5),
                                 side="right"))
                att['part'] = sR.enter_context(
                    tc.tile_pool(name="part",
                                 bufs=(3 if 'part3' in FLAGS else 2),
                                 side="right"))
                att['rb'] = sR.enter_context(
                    tc.tile_pool(name="rb",
                                 bufs=(3 if 'rb3' in FLAGS else 2),
                                 side="right"))
                att['psA'] = sR.enter_context(
                    tc.tile_pool(name="psA", bufs=1, space="PSUM"))

            expTs = {}
            parts = {}
            rbs = {}

            def emit_partial(b, hc):
                """DVE pre-reduce of the kv-chunk blocks into partial[128,T]
                (softmax denominator before the cross-partition reduce).
                Emitted only once exp data is near-ready so the in-order DVE
                queue is not head-of-line blocked."""
                expT = expTs[(b, hc)]
                part = att['part'].tile([P, T], f16, tag="part",
                                        name=f"part{b}_{hc}")
                parts[(b, hc)] = part
                nc.vector.tensor_copy(part[:], expT[:, offs[0]:offs[0] + T])
                with nc.allow_low_precision(
                        reason="fp16 partial rowsums; d<=~3e3, tol 2e-2"):
                    for c in range(1, TC8):
                        w = T - c * P
                        nc.vector.tensor_add(
                            part[:, c * P:T], part[:, c * P:T],
                            expT[:, offs[c]:offs[c] + w])

            def emit_dps(b, hc):
                """softmax denominator: ones-matmul does the cross-partition
                reduce AND the 128-way broadcast; DVE reciprocal to SBUF."""
                psA = att['psA']
                part = parts.pop((b, hc))
                rb = att['rb'].tile([P, T], f16, tag="rb", name=f"rb{b}_{hc}")
                rbs[(b, hc)] = rb
                for (q0, q1) in ((0, 512), (512, T)):
                    dps = psA.tile([P, 512], f32, tag="acc",
                                   bufs=(6 if 'acc6' in FLAGS else 4),
                                   name=f"dps{b}_{hc}_{q0}")
                    nc.tensor.matmul(dps[:], ones2d[:], part[:, q0:q1],
                                     start=True, stop=True)
                    with nc.allow_low_precision(
                            reason="fp16 1/d; d in [1,~3e3], tol 2e-2"):
                        nc.vector.reciprocal(rb[:, q0:q1], dps[:])

            def emit_scores(b, hc):
                """scoresT = kT.T @ qT per kv chunk, exp on ACT, causal mask
                on GpSimd, then DVE pre-reduce of the kv-chunk blocks into
                partial[128, T] (the softmax denominator, pre cross-part)."""
                psA = att['psA']
                tok = slice(b * T, (b + 1) * T)
                qT_i = q16[:, hc, tok]
                expT = att['expT'].tile([P, acc], f16, tag="expT",
                                        name=f"expT{b}_{hc}")
                expTs[(b, hc)] = expT
                for c in range(TC8):
                    kT_c = k16[:, b * T + c * P: b * T + (c + 1) * P]
                    spans = [(c * P, 512)] if c < 4 else []
                    spans += [(max(512, c * P), T)]
                    for si, (q0, q1) in enumerate(spans):
                        sps = psA.tile([P, 512], f32, tag="acc",
                                       bufs=(6 if 'acc6' in FLAGS else 4),
                                       name=f"sps{b}_{hc}_{c}_{q0}")
                        w = q1 - q0
                        nc.tensor.matmul(sps[:, :w], kT_c,
                                         qT_i[:, q0:q1],
                                         start=True, stop=True)
                        eo = offs[c] + (q0 - c * P)
                        nc.scalar.activation(
                            expT[:, eo:eo + w], sps[:, :w],
                            mybir.ActivationFunctionType.Exp, scale=SCALE)
                    # zero the invalid (kv > q) half of the diagonal block
                    meng = nc.vector if 'mask_dve' in FLAGS else nc.gpsimd
                    meng.tensor_mul(
                        expT[:, offs[c]:offs[c] + P],
                        expT[:, offs[c]:offs[c] + P], maskT[:])
                if not (b == 1 and 'late_part' in FLAGS):
                    emit_partial(b, hc)

            def emit_pv(b, hc):
                """y = probs @ v (unnormalized), then denominator
                reduce+broadcast via ones-matmul, DVE reciprocal, and the
                normalizing multiply fused into the PSUM->SBUF move."""
                psA = att['psA']
                tok = slice(b * T, (b + 1) * T)
                if (b, hc) not in parts and (b, hc) not in rbs:
                    emit_partial(b, hc)
                expT = expTs.pop((b, hc))
                if 'ysplit' in FLAGS:
                    ypss = {}
                    for (s0, s1) in ((0, 512), (512, T)):
                        ypss[s0] = psA.tile(
                            [P, 512], f32, tag="acc",
                            bufs=(6 if 'acc6' in FLAGS else 4),
                            name=f"yps{b}_{hc}_{s0}")
                        cs = [c for c in range(TC8) if c * P < s1]
                        for c in cs:
                            q0 = max(s0, c * P)
                            sl = slice(offs[c] + (q0 - c * P),
                                       offs[c] + (s1 - c * P))
                            nc.tensor.matmul(
                                ypss[s0][:, q0 - s0:s1 - s0],
                                vtm[:, b * TC8 + c, :],
                                expT[:, sl], start=(c == 0),
                                stop=(c == cs[-1]))
                    if (b, hc) not in rbs:
                        emit_dps(b, hc)
                    rb = rbs.pop((b, hc))
                    ymeng = nc.vector if 'ymul_dve' in FLAGS else nc.gpsimd
                    for (s0, s1) in ((0, 512), (512, T)):
                        ymeng.tensor_mul(
                            y_sb[:, hc, b * T + s0:b * T + s1],
                            ypss[s0][:], rb[:, s0:s1])
                else:
                    yps = att.get('psY', psA).tile(
                        [P, T], f32, tag="yps",
                        bufs=(1 if ('yps1' in FLAGS or 'acc6' in FLAGS) else 2),
                        name=f"yps{b}_{hc}")
                    for (s0, s1) in ((0, 512), (512, T)):
                        cs = [c for c in range(TC8) if c * P < s1]
                        for c in cs:
                            q0 = max(s0, c * P)
                            sl = slice(offs[c] + (q0 - c * P),
                                       offs[c] + (s1 - c * P))
                            nc.tensor.matmul(
                                yps[:, q0:s1], vtm[:, b * TC8 + c, :],
                                expT[:, sl], start=(c == 0), stop=(c == cs[-1]))
                    if (b, hc) not in rbs:
                        emit_dps(b, hc)
                    rb = rbs.pop((b, hc))
                    (nc.vector if 'ymul_dve' in FLAGS
                     else nc.gpsimd).tensor_mul(
                        y_sb[:, hc, tok], yps[:], rb[:])

            def emit_proj(m):
                """out[tokens m*128:(m+1)*128, :] = y.T @ wproj (partial)."""
                psA = att['psA']
                ob = ob_pool.tile([P, NE], f16, tag="ob", name=f"ob{m}")
                for n in range(NE // 512):
                    opsum = psA.tile([P, 512], f32, tag="acc",
                                     bufs=(6 if 'acc6' in FLAGS else 4),
                                     name=f"ops{m}_{n}")
                    for kc in range(QPK):
                        nc.tensor.matmul(
                            opsum[:], y_sb[:, kc, m * P:(m + 1) * P],
                            wp[:, kc, n * 512:(n + 1) * 512],
                            start=(kc == 0), stop=(kc == QPK - 1))
                    if 'tail_alt' in FLAGS and m == NT // P - 1:
                        if n % 2:
                            nc.vector.tensor_copy(
                                ob[:, n * 512:(n + 1) * 512], opsum[:])
                        else:
                            nc.scalar.copy(
                                ob[:, n * 512:(n + 1) * 512], opsum[:])
                    else:
                        (nc.vector if ('route_copies' in FLAGS
                                       or 'obcopy_dve' in FLAGS)
                         else nc.any).tensor_copy(
                            ob[:, n * 512:(n + 1) * 512], opsum[:])
                    if 'tail_split' in FLAGS and m == NT // P - 1:
                        c0, c1 = n * 512, (n + 1) * 512
                        nc.sync.dma_start(
                            out_d[m * P:(m + 1) * P, c0:c1], ob[:, c0:c1])
                    elif 'quarter_dma' in FLAGS:
                        if n % 2 == 1:
                            c0, c1 = (n - 1) * 512, (n + 1) * 512
                            nc.sync.dma_start(
                                out_d[m * P:(m + 1) * P, c0:c1], ob[:, c0:c1])
                    elif n == 3:
                        nc.sync.dma_start(
                            out_d[m * P:(m + 1) * P, 0:2048], ob[:, 0:2048])
                    elif n == 7:
                        nc.sync.dma_start(
                            out_d[m * P:(m + 1) * P, 2048:NE], ob[:, 2048:NE])

            # ============ phase 1+2: qkv projection + rope, per batch ========
            with ExitStack() as sA:
                qkv_pool = sA.enter_context(tc.tile_pool(name="qkv", bufs=1))
                qkv = qkv_pool.tile([P, MC, NT], f16)
                wq_pool = sA.enter_context(tc.tile_pool(name="wq", bufs=1))
                wq = wq_pool.tile([P, KC, GW], f16)
                wqr = wqkvT_d[:].rearrange("(ko p) m -> p ko m", p=P)
                xs_pool = sA.enter_context(tc.tile_pool(
                    name="xs",
                    bufs=(16 if 'xs16' in FLAGS else 8 if 'xs8' in FLAGS else 4)))
                rp = sA.enter_context(tc.tile_pool(
                    name="rope",
                    bufs=(4 if 'rope4' in FLAGS else 3 if 'rope3' in FLAGS
                          else 2)))

                cpeng = (nc.vector
                         if ('route_copies' in FLAGS or 'qkvcopy_dve' in FLAGS)
                         else nc.any)
                cpeng0 = nc.vector if 'qkvcopy0_dve' in FLAGS else cpeng

                def rope_span(b, tok, w):
                    h = HS // 2
                    ccb, ssb = cc[:, tok], ss[:, tok]
                    order = list(range(QPK + 1))
                    if 'rope_korder' in FLAGS and b == 1:
                        order = [QPK] + list(range(QPK))
                    # half-spans get their own tags: mixed tile sizes inside
                    # one rotation tag alias SBUF and corrupt data on HW
                    sfx = "" if w == T else "H"
                    for hc in order:
                        src_ = qkv[:, hc, tok]
                        rot = rp.tile([P, w], f16, tag="rot" + sfx,
                                      name=f"rot{b}_{hc}_{tok.start}")
                        nc.sync.dma_start(rot[0:h, :], src_[h:P, :])
                        nc.sync.dma_start(rot[h:P, :], src_[0:h, :])
                        t1 = rp.tile([P, w], f16, tag="t1" + sfx,
                                     name=f"t1_{b}_{hc}_{tok.start}")
                        t2 = rp.tile([P, w], f16, tag="t2" + sfx,
                                     name=f"t2_{b}_{hc}_{tok.start}")
                        reng = (nc.gpsimd if ('rope_pool' in FLAGS or
                                ('rope1_pool' in FLAGS and b == 1) or
                                ('rope_tail_pool' in FLAGS and b == 1
                                 and 2 <= hc < QPK))
                                else nc.vector)
                        reng.tensor_mul(t1[:], src_, ccb)
                        reng.tensor_mul(t2[:], rot[:], ssb)
                        dst = (q16[:, hc, tok] if hc < QPK
                               else k16[:, tok])
                        with nc.allow_low_precision(
                                reason="fp16 rope; |q|,|k|~1, tol 2e-2"):
                            reng.tensor_add(dst, t1[:], t2[:])

                def rope_batch(b):
                    tok = slice(b * T, (b + 1) * T)
                    rope_span(b, tok, T)

                def rope_nhalf(n):
                    rope_span(n // 2, slice(n * 512, (n + 1) * 512), 512)

                def vt_batch(b, pool, tag, bufs, shape, cs=None):
                    for c in (range(TC8) if cs is None else cs):
                        # PE transpose (avoids XBAR DMA-transpose, which
                        # races concurrent DMA copies on this stack)
                        vt_ps = pool.tile(shape, f16, tag=tag, bufs=bufs,
                                          name=f"vt{b}_{c}")
                        nc.tensor.transpose(
                            vt_ps[:, 0:P],
                            qkv[:, QPK + 1, b * T + c * P: b * T + (c + 1) * P],
                            ident16[:])
                        (nc.vector if ('route_copies' in FLAGS
                                       or 'vtcopy_dve' in FLAGS)
                         else nc.any).tensor_copy(
                            vtm[:, b * TC8 + c, :], vt_ps[:, 0:P])

                def ident_dma():
                    nc.sync.dma_start(ident16[:], ident16_d[:])

                def consts_dma():
                    nc.sync.dma_start(cc[:], cc_d[:])
                    nc.sync.dma_start(ss[:], ss_d[:])
                    nc.sync.dma_start(maskT[:], mask_d[:])
                    nc.sync.dma_start(negtri[:], negtri_d[:])
                    nc.sync.dma_start(ones2d[:], ones2d_d[:])

                if ILV:
                    # resident xt set shared by both batches (tag bufs=32)
                    def load_xt(n, k):
                        xt = xs_pool.tile([P, 512], f16, tag="xtr",
                                          bufs=32, name=f"xt{n}_{k}")
                        if n == 0 and k == 0 and 'xt0_split' in FLAGS:
                            nc.sync.dma_start(xt[:, 0:256],
                                              xT_d[0:P, 0:256])
                            nc.sync.dma_start(xt[:, 256:512],
                                              xT_d[0:P, 256:512])
                        else:
                            nc.sync.dma_start(
                                xt[:], xT_d[k * P:(k + 1) * P,
                                            n * 512:(n + 1) * 512])
                        return xt

                    # ---- batch 0: flat 6-psum sweeps, own PSUM pool ----
                    with ExitStack() as sP0:
                        ps1a = sP0.enter_context(
                            tc.tile_pool(name="ps1a", bufs=6, space="PSUM"))
                        for n in (0, 1):
                            psums = [ps1a.tile([P, 512], f32, tag="ps1",
                                               name=f"ps1_{n}_{m_}")
                                     for m_ in range(MC)]
                            for k in range(KC):
                                if n == 0:
                                    if k == 0 and 'wq_split' in FLAGS:
                                        nc.sync.dma_start(wq[:, 0, 0:P],
                                                          wqr[:, 0, 0:P])
                                        nc.sync.dma_start(wq[:, 0, P:GW],
                                                          wqr[:, 0, P:GW])
                                    else:
                                        nc.sync.dma_start(wq[:, k, :],
                                                          wqr[:, k, :])
                                xt = load_xt(n, k)
                                for m in range(MC):
                                    nc.tensor.matmul(
                                        psums[m][:],
                                        wq[:, k, m * P:(m + 1) * P],
                                        xt[:], start=(k == 0),
                                        stop=(k == KC - 1))
                            for m in range(MC):
                                if 'altcopy' in FLAGS and m % 2 == 0:
                                    nc.scalar.copy(
                                        qkv[:, m, n * 512:(n + 1) * 512],
                                        psums[m][:])
                                elif 'altcopy' in FLAGS:
                                    nc.vector.tensor_copy(
                                        qkv[:, m, n * 512:(n + 1) * 512],
                                        psums[m][:])
                                else:
                                    cpeng0.tensor_copy(
                                        qkv[:, m, n * 512:(n + 1) * 512],
                                        psums[m][:])
                            if n == 0:
                                # MUST precede the first vt transpose: a
                                # reader emitted before its producer DMA
                                # gets no dependency and reads uninitialized
                                # SBUF (ident16 is vt's identity operand)
                                ident_dma()
                                if 'rope_half' in FLAGS:
                                    consts_dma()
                            if 'rope_half' in FLAGS:
                                rope_nhalf(n)
                            if 'vt_split' in FLAGS:
                                vt_batch(0, ps1a, "vt", 2, [P, P],
                                         cs=range(n * 4, n * 4 + 4))
                        if 'rope_half' not in FLAGS:
                            # before rope (cc/ss) and the attention consts
                            consts_dma()
                            rope_batch(0)
                        if 'vt_split' not in FLAGS:
                            vt_batch(0, ps1a, "vt", 2, [P, P])

                    # attention pools come alive before batch 1 so batch-0
                    # scores/exp interleave into batch-1's qkv stream
                    make_att_pools()

                    # ---- batch 1: two 3-psum m-groups per n-chunk ----
                    if 'acc6' in FLAGS:
                        groups = ((0, 1), (2, 3), (4, 5))
                        ps1b_bufs, ngr = 2, 3
                    else:
                        groups = ((0, 1, 2), (3, 4, 5))
                        ps1b_bufs, ngr = 4, 2
                    with ExitStack() as sP1:
                        ps1b = sP1.enter_context(
                            tc.tile_pool(name="ps1b", bufs=ps1b_bufs,
                                         space="PSUM"))
                        for n in (2, 3):
                            xts = {}
                            for g, ms in enumerate(groups):
                                psums = {m_: ps1b.tile(
                                    [P, 512], f32, tag="ps1",
                                    name=f"ps1_{n}_{g}_{m_}") for m_ in ms}
                                for k in range(KC):
                                    if g == 0:
                                        xts[k] = load_xt(n, k)
                                    for m in ms:
                                        nc.tensor.matmul(
                                            psums[m][:],
                                            wq[:, k, m * P:(m + 1) * P],
                                            xts[k][:], start=(k == 0),
                                            stop=(k == KC - 1))
                                for m in ms:
                                    cpeng.tensor_copy(
                                        qkv[:, m, n * 512:(n + 1) * 512],
                                        psums[m][:])
                                if (('rope_half' in FLAGS
                                     or 'rope_half1' in FLAGS)
                                        and g == len(groups) - 1):
                                    rope_nhalf(n)
                                slot = (n - 2) * ngr + g
                                if slot < (QPK if 'acc6' in FLAGS else 3):
                                    emit_scores(0, slot)
                                if ('early_dps' in FLAGS and
                                        1 <= slot <= QPK):
                                    emit_dps(0, slot - 1)
                                if 'pv_in_p1' in FLAGS and slot >= QPK:
                                    emit_pv(0, slot - QPK)
                        if ('rope_half' not in FLAGS
                                and 'rope_half1' not in FLAGS):
                            rope_batch(1)
                        if 'acc6' not in FLAGS:
                            emit_scores(0, 3)
                        vt_batch(1, att['psA'], "acc",
                                 (6 if 'acc6' in FLAGS else 4), [P, 1024])
                else:
                    ps1 = sA.enter_context(
                        tc.tile_pool(name="ps1", bufs=6, space="PSUM"))
                    for b in range(B):
                        for n in (2 * b, 2 * b + 1):
                            psums = [ps1.tile([P, 512], f32, tag="ps1",
                                              name=f"ps1_{n}_{m_}")
                                     for m_ in range(MC)]
                            for k in range(KC):
                                if n == 0:
                                    if k == 0 and 'wq_split' in FLAGS:
                                        nc.sync.dma_start(wq[:, 0, 0:P],
                                                          wqr[:, 0, 0:P])
                                        nc.sync.dma_start(wq[:, 0, P:GW],
                                                          wqr[:, 0, P:GW])
                                    else:
                                        nc.sync.dma_start(wq[:, k, :],
                                                          wqr[:, k, :])
                                xt = xs_pool.tile([P, 512], f16, tag="xt",
                                                  name=f"xt{n}_{k}")
                                nc.sync.dma_start(
                                    xt[:], xT_d[k * P:(k + 1) * P,
                                                n * 512:(n + 1) * 512])
                                for m in range(MC):
                                    nc.tensor.matmul(
                                        psums[m][:],
                                        wq[:, k, m * P:(m + 1) * P],
                                        xt[:], start=(k == 0),
                                        stop=(k == KC - 1))
                            for m in range(MC):
                                cpeng.tensor_copy(
                                    qkv[:, m, n * 512:(n + 1) * 512],
                                    psums[m][:])
                        if b == 0:
                            ident_dma()
                            consts_dma()
                        rope_batch(b)
                        vt_batch(b, ps1, "vt", 2, [P, P])

            if not ('early_pools' in FLAGS or 'pv_in_p1' in FLAGS):
                y_sb, wp = alloc_yw_pools()
            ob_pool = sL.enter_context(tc.tile_pool(name="ob", bufs=(3 if 'ob3' in FLAGS else 2)))
            for kc in range(QPK):
                nc.sync.dma_start(wp[:, kc, :], wpr[:, kc, :])

            # ============ phases 3+4 pools ============
            if not ILV:
                make_att_pools()
            elif 'ysplit' not in FLAGS:
                att['psY'] = sR.enter_context(
                    tc.tile_pool(name="psY", bufs=1, space="PSUM"))

            # ===== batch 0 attention =====
            if ILV and 'acc6' in FLAGS:
                if 'dps_first' in FLAGS:
                    for h_ in range(QPK):
                        emit_dps(0, h_)
                if 'dps1ahead' in FLAGS:
                    emit_pv(0, 0)
                    emit_dps(0, 1)
                    emit_scores(1, 0)
                    emit_pv(0, 1)
                    emit_dps(0, 2)
                    emit_scores(1, 1)
                    emit_pv(0, 2)
                    emit_dps(0, 3)
                    emit_scores(1, 2)
                    emit_pv(0, 3)
                    emit_scores(1, 3)
                    seq = []
                elif 'pv_in_p1' in FLAGS:
                    seq = [('pv', 2), ('sc', 0), ('pv', 3), ('sc', 1)]
                elif 'bord1' in FLAGS:
                    seq = [('pv', 0), ('sc', 0), ('sc', 1), ('pv', 1),
                           ('sc', 2), ('pv', 2), ('sc', 3), ('pv', 3)]
                elif 'bord2' in FLAGS:
                    seq = [('pv', 0), ('sc', 0), ('sc', 1), ('pv', 1),
                           ('pv', 2), ('sc', 2), ('sc', 3), ('pv', 3)]
                else:
                    seq = [('pv', 0), ('sc', 0), ('pv', 1), ('sc', 1),
                           ('pv', 2), ('sc', 2), ('pv', 3), ('sc', 3)]
                for op, i in seq:
                    if op == 'pv':
                        emit_pv(0, i)
                    else:
                        emit_scores(1, i)
            elif ILV:
                emit_pv(0, 0)
                emit_pv(0, 1)
                emit_scores(1, 0)
                emit_pv(0, 2)
                emit_pv(0, 3)
            else:
                for hc in range(QPK):
                    emit_scores(0, hc)
                emit_pv(0, 0)
                emit_pv(0, 1)
                emit_scores(1, 0)      # early: fills PE while ACT drains b0
                emit_pv(0, 2)
                emit_pv(0, 3)

            # ===== batch 1 attention interleaved with batch-0 proj: proj
            # matmuls keep PE busy while ACT runs exp for the next head =====
            if 'acc6' in FLAGS and 'pv_in_p1' in FLAGS:
                emit_proj(0)
                emit_scores(1, 2)
                emit_partial(1, 0)
                emit_proj(1)
                emit_scores(1, 3)
                emit_partial(1, 1)
                emit_proj(2)
                emit_pv(1, 0)
                emit_proj(3)
                emit_partial(1, 2)
                emit_pv(1, 1)
                emit_proj(4)
                emit_partial(1, 3)
                emit_proj(5)
                emit_pv(1, 2)
                emit_proj(6)
                emit_pv(1, 3)
                emit_proj(7)
            elif 'acc6' in FLAGS:
                if 'b1ord1' in FLAGS:
                    plan = [('pt', 0), ('pj', 0), ('pt', 1), ('pv', 0),
                            ('pj', 1), ('pt', 2), ('pv', 1), ('pj', 2),
                            ('pt', 3), ('pv', 2), ('pj', 3), ('pv', 3),
                            ('pj', 4), ('pj', 5), ('pj', 6), ('pj', 7)]
                elif 'b1ord2' in FLAGS:
                    plan = [('pt', 0), ('pj', 0), ('pj', 1), ('pt', 1),
                            ('pv', 0), ('pj', 2), ('pt', 2), ('pj', 3),
                            ('pv', 1), ('pj', 4), ('pt', 3), ('pv', 2),
                            ('pj', 5), ('pv', 3), ('pj', 6), ('pj', 7)]
                else:
                    plan = [('pt', 0), ('pj', 0), ('pt', 1), ('pv', 0),
                            ('pj', 1), ('pt', 2), ('pj', 2), ('pv', 1),
                            ('pj', 3), ('pt', 3), ('pj', 4), ('pv', 2),
                            ('pj', 5), ('pj', 6), ('pv', 3), ('pj', 7)]
                for op, i in plan:
                    if op == 'pt':
                        if 'late_part' in FLAGS:
                            emit_partial(1, i)
                    elif op == 'pv':
                        emit_pv(1, i)
                    else:
                        emit_proj(i)
            else:
                emit_proj(0)
                emit_pv(1, 0)
                emit_scores(1, 1)
                emit_proj(1)
                emit_proj(2)
                emit_pv(1, 1)
                emit_scores(1, 2)
                emit_proj(3)
                emit_proj(4)
                emit_pv(1, 2)
                emit_scores(1, 3)
                emit_proj(5)
                emit_proj(6)
                emit_pv(1, 3)
                emit_proj(7)
            for m in range(8, NT // P):
                emit_proj(m)
        finally:
            sR.close()
            sL.close()

    if split_waits:
        _split_waits(nc, mybir)
    return nc


def _host_prep(x, cos, sin, W_attn, W_proj):
    xT = np.ascontiguousarray(x.reshape(NT, NE).T.astype(np.float16))
    cosT = np.tile(cos.T, (1, B))
    sinT = np.tile(sin.T, (1, B))
    cc = np.ascontiguousarray(
        np.concatenate([cosT, cosT], axis=0), dtype=np.float16)
    ss = np.ascontiguousarray(
        np.concatenate([-sinT, sinT], axis=0), dtype=np.float16)
    # scoresT layout [kv, q]: zero strictly-lower (kv > q) entries post-exp
    maskT = np.triu(np.ones((P, P), dtype=np.float16))
    negtri = np.where(np.triu(np.ones((P, P), dtype=bool)), 0.0,
                      -1.0e30).astype(np.float32)
    common = {"xT": xT, "cc": cc, "ss": ss, "maskT": maskT, "negtri": negtri,
              "ident16": np.eye(P, dtype=np.float16),
              "ones2d": np.ones((P, P), dtype=np.float16)}
    in_maps = []
    for g in range(NCORES):
        m = dict(common)
        m["wqkvT"] = np.ascontiguousarray(
            W_attn[g * GW:(g + 1) * GW, :].T.astype(np.float16))
        m["wprojT"] = np.ascontiguousarray(
            W_proj[:, g * GQ:(g + 1) * GQ].T.astype(np.float16))
        in_maps.append(m)
    return in_maps


LAST_EXEC_NS = None


def kernel(x, cos, sin, W_attn, W_proj, max_seq_length):
    global LAST_EXEC_NS
    from concourse.bass_utils import run_bass_kernel_spmd

    x = np.asarray(x, dtype=np.float32)
    cos = np.asarray(cos, dtype=np.float32)
    sin = np.asarray(sin, dtype=np.float32)
    W_attn = np.asarray(W_attn, dtype=np.float32)
    W_proj = np.asarray(W_proj, dtype=np.float32)

    if "nc" not in _CACHE:
        _CACHE["nc"] = _build_nc()
    nc = _CACHE["nc"]

    in_maps = _host_prep(x, cos, sin, W_attn, W_proj)
    res = run_bass_kernel_spmd(nc, in_maps, core_ids=list(range(NCORES)))
    LAST_EXEC_NS = res.exec_time_ns

    acc = res.results[0]["out"].astype(np.float32)
    for g in range(1, NCORES):
        acc = acc + res.results[g]["out"].astype(np.float32)
    return acc.reshape(B, T, NE)



# revision 4
# speedup vs baseline: 1.1405x; 1.1405x over previous
"""Trainium2 Bass kernel for CausalSelfAttention (GQA, RoPE, prefill).

Tensor-parallel over the 8 query groups: core g owns query heads
[4g, 4g+4) and kv head g.  Each core computes a partial output
(full-shape, fp16) that the host sums in fp32.

The two dense projections (qkv: x@W_attn slice, proj: y@W_proj slice)
run as 3-term error-compensated fp8 DoubleRow matmuls:
    x @ W  ~=  x_hi@W_hi + x_lo@W_hi + x_hi@W_lo
with x_hi = e4m3(x), x_lo = e4m3(x - x_hi) (same for W, pre-scaled by
32 on host so W's ~N(0, 1/4096) entries stay in e4m3 normal range).
DoubleRow packs two (128-contraction-plane, term) pairs per PE
instruction at 0.5 cycles per output column, so each term costs 1/4 of
an fp16 matmul and the compensated product runs at 0.75x fp16 time with
~0.2% error (vs ~4% for naive fp8).  Scale bookkeeping: q,k are
descaled by folding 1/32 into the host cos/sin tables; v stays 32x and
W_proj adds another 32x, so the host divides the summed output by 1024.

Attention stays fp16: scores KV-MAJOR (scoresT = kT.T @ qT, 6-deep PSUM
rotation), exp on ACT straight into the PV rhs layout, causal-diagonal
mask on GpSimd, softmax denominator via DVE block pre-reduce + one
ones-matmul (cross-partition reduce + broadcast) + DVE reciprocal.
emit_pv additionally splits y into fp8 hi/lo for the proj stage
(DVE mul, ACT quantize-copy, GpSimd residual-subtract).

Schedule skeleton (every engine stream is in-order, so EMISSION ORDER
is the schedule): batch-0 qkv runs flat 6-PSUM kp-sweeps; batch 1 runs
three 2-PSUM m-groups per n-chunk with batch-0 attention interleaved;
batch-0 proj chunks interleave into batch-1's attention as PE filler.
"""

import os
import numpy as np

FLAGS = set(os.environ.get(
    'KFLAGS',
    'quarter_dma,tail_split,tail_alt,altcopy,late_part,vt_split').split(','))

B, T, NE, NH, NQG, HS = 2, 1024, 4096, 32, 8, 128
QPK = NH // NQG          # 4 query heads per kv group
NT = B * T               # 2048 tokens
GW = (QPK + 2) * HS      # 768 qkv rows per group
GQ = QPK * HS            # 512 q cols per group
P = 128
NCORES = 8
KC = NE // P             # 32 contraction chunks for qkv proj
KP = KC // 2             # 16 DoubleRow plane-pairs
MC = GW // P             # 6 qkv feature chunks
TC8 = T // P             # 8 token chunks per batch
NNC = NT // 512          # 4 token n-chunks
SCALE = 1.0 / float(np.sqrt(HS))
WSCALE = 32.0            # host pre-scale on W_attn / W_proj before e4m3

_CACHE = {}


def _split_waits(nc, mybir, max_waits=1):
    """walrus in this container rejects >1 sync-wait per instruction;
    hoist extras onto single-wait NoOps just before (equivalent since
    semaphores are monotonic and a sequencer executes in order)."""
    for fn in nc.m.functions:
        for blk in fn.blocks:
            new_list, changed = [], False
            for inst in blk.instructions:
                si = getattr(inst, "sync_info", None)
                if si is not None and len(si.on_wait) > max_waits:
                    waits = list(si.on_wait)
                    for i, w in enumerate(waits[:-max_waits]):
                        nop = mybir.InstNoOp(
                            name=f"{inst.name}-wsplit-{i}", ins=[], outs=[],
                            engine=inst.engine)
                        nop.sync_info = mybir.SyncInfo(on_wait=[w], on_update=[])
                        new_list.append(nop)
                    inst.sync_info = mybir.SyncInfo(
                        on_wait=waits[-max_waits:], on_update=list(si.on_update))
                    changed = True
                new_list.append(inst)
            if changed:
                blk.instructions = new_list


def _build_nc(reps=1, split_waits=True):
    import concourse.bass as bass
    import concourse.mybir as mybir
    import concourse.tile as tile
    from contextlib import ExitStack

    f32 = mybir.dt.float32
    f16 = mybir.dt.float16
    f8 = mybir.dt.float8e4
    DR = mybir.MatmulPerfMode.DoubleRow

    nc = bass.Bass()
    # fp8 pair-packed inputs (see _host_prep for the exact layouts)
    x8h_d = nc.dram_tensor("x8h", [NNC, KP, P, 2, 512], f8, kind="ExternalInput")
    x8l_d = nc.dram_tensor("x8l", [NNC, KP, P, 2, 512], f8, kind="ExternalInput")
    wqh_d = nc.dram_tensor("wqh", [KP, P, MC, 2, P], f8, kind="ExternalInput")
    wql_d = nc.dram_tensor("wql", [KP, P, MC, 2, P], f8, kind="ExternalInput")
    wph_d = nc.dram_tensor("wph", [2, P, NE // 512, 2, 512], f8,
                           kind="ExternalInput")
    wpl_d = nc.dram_tensor("wpl", [2, P, NE // 512, 2, 512], f8,
                           kind="ExternalInput")
    cc_d = nc.dram_tensor("cc", [P, NT], f16, kind="ExternalInput")
    ss_d = nc.dram_tensor("ss", [P, NT], f16, kind="ExternalInput")
    mask_d = nc.dram_tensor("maskT", [P, P], f16, kind="ExternalInput")
    ones2d_d = nc.dram_tensor("ones2d", [P, P], f16, kind="ExternalInput")
    ident16_d = nc.dram_tensor("ident16", [P, P], f16, kind="ExternalInput")
    out_d = nc.dram_tensor("out", [NT, NE], f16, kind="ExternalOutput")

    # column offset of kv-chunk c's block inside the expT tile
    offs, acc = [], 0
    for c in range(TC8):
        offs.append(acc)
        acc += (TC8 - c) * P

    with tile.TileContext(nc) as tc:
      for _rep in range(reps):
        sL = ExitStack()   # left-side long-lived pools (y8, wp8, ob)
        sR = ExitStack()   # right-side pools (qk16, attention-era)
        try:
            # const: left
            const = sL.enter_context(tc.tile_pool(name="const", bufs=1))
            cc = const.tile([P, NT], f16)
            ss = const.tile([P, NT], f16)
            maskT = const.tile([P, P], f16)
            ones2d = const.tile([P, P], f16)
            ident16 = const.tile([P, P], f16)

            def alloc_yw_pools():
                y_pool = sL.enter_context(tc.tile_pool(name="y", bufs=1))
                # (p, kp-pair, token-chunk, slot, col) — proj lhsT slices
                y8h = y_pool.tile([P, 2, NT // P, 2, P], f8)
                y8l = y_pool.tile([P, 2, NT // P, 2, P], f8)
                wp_pool = sL.enter_context(tc.tile_pool(name="wp", bufs=1))
                wp8h = wp_pool.tile([P, 2, NE // 512, 2, 512], f8)
                wp8l = wp_pool.tile([P, 2, NE // 512, 2, 512], f8)
                return y8h, y8l, wp8h, wp8l

            # qk16 on the right: lives through attention
            qk16 = sR.enter_context(tc.tile_pool(name="qk16", bufs=1, side="right"))
            q16 = qk16.tile([P, QPK, NT], f16)
            k16 = qk16.tile([P, NT], f16)
            vtm = qk16.tile([P, B * TC8, P], f16)

            att = {}

            def make_att_pools():
                att['expT'] = sR.enter_context(
                    tc.tile_pool(name="expT", bufs=5, side="right"))
                att['part'] = sR.enter_context(
                    tc.tile_pool(name="part", bufs=2, side="right"))
                att['rb'] = sR.enter_context(
                    tc.tile_pool(name="rb", bufs=2, side="right"))
                att['y16'] = sR.enter_context(
                    tc.tile_pool(name="y16", bufs=2, side="right"))
                att['psA'] = sR.enter_context(
                    tc.tile_pool(name="psA", bufs=1, space="PSUM"))

            expTs = {}
            parts = {}
            rbs = {}

            def emit_partial(b, hc):
                """DVE pre-reduce of the kv-chunk blocks into partial[128,T]
                (softmax denominator before the cross-partition reduce).
                Emitted only once exp data is near-ready so the in-order DVE
                queue is not head-of-line blocked."""
                expT = expTs[(b, hc)]
                part = att['part'].tile([P, T], f16, tag="part",
                                        name=f"part{b}_{hc}")
                parts[(b, hc)] = part
                nc.vector.tensor_copy(part[:], expT[:, offs[0]:offs[0] + T])
                with nc.allow_low_precision(
                        reason="fp16 partial rowsums; d<=~3e3, tol 2e-2"):
                    for c in range(1, TC8):
                        w = T - c * P
                        nc.vector.tensor_add(
                            part[:, c * P:T], part[:, c * P:T],
                            expT[:, offs[c]:offs[c] + w])

            def emit_dps(b, hc):
                """softmax denominator: ones-matmul does the cross-partition
                reduce AND the 128-way broadcast; DVE reciprocal to SBUF."""
                psA = att['psA']
                part = parts.pop((b, hc))
                rb = att['rb'].tile([P, T], f16, tag="rb", name=f"rb{b}_{hc}")
                rbs[(b, hc)] = rb
                for (q0, q1) in ((0, 512), (512, T)):
                    dps = psA.tile([P, 512], f32, tag="acc", bufs=6,
                                   name=f"dps{b}_{hc}_{q0}")
                    nc.tensor.matmul(dps[:], ones2d[:], part[:, q0:q1],
                                     start=True, stop=True)
                    with nc.allow_low_precision(
                            reason="fp16 1/d; d in [1,~3e3], tol 2e-2"):
                        nc.vector.reciprocal(rb[:, q0:q1], dps[:])

            def emit_scores(b, hc):
                """scoresT = kT.T @ qT per kv chunk, exp on ACT, causal mask
                on GpSimd."""
                psA = att['psA']
                tok = slice(b * T, (b + 1) * T)
                qT_i = q16[:, hc, tok]
                expT = att['expT'].tile([P, acc], f16, tag="expT",
                                        name=f"expT{b}_{hc}")
                expTs[(b, hc)] = expT
                for c in range(TC8):
                    kT_c = k16[:, b * T + c * P: b * T + (c + 1) * P]
                    spans = [(c * P, 512)] if c < 4 else []
                    spans += [(max(512, c * P), T)]
                    for si, (q0, q1) in enumerate(spans):
                        sps = psA.tile([P, 512], f32, tag="acc", bufs=6,
                                       name=f"sps{b}_{hc}_{c}_{q0}")
                        w = q1 - q0
                        nc.tensor.matmul(sps[:, :w], kT_c,
                                         qT_i[:, q0:q1],
                                         start=True, stop=True)
                        eo = offs[c] + (q0 - c * P)
                        nc.scalar.activation(
                            expT[:, eo:eo + w], sps[:, :w],
                            mybir.ActivationFunctionType.Exp, scale=SCALE)
                    # zero the invalid (kv > q) half of the diagonal block
                    nc.gpsimd.tensor_mul(
                        expT[:, offs[c]:offs[c] + P],
                        expT[:, offs[c]:offs[c] + P], maskT[:])
                if not (b == 1 and 'late_part' in FLAGS):
                    emit_partial(b, hc)

            def emit_pv(b, hc):
                """y = probs @ v (unnormalized), denominator reduce+broadcast
                via ones-matmul, DVE reciprocal, normalizing multiply into a
                fp16 scratch, then fp8 hi/lo split for the proj stage."""
                psA = att['psA']
                if (b, hc) not in parts and (b, hc) not in rbs:
                    emit_partial(b, hc)
                expT = expTs.pop((b, hc))
                yps = att['psY'].tile([P, T], f32, tag="yps", bufs=1,
                                      name=f"yps{b}_{hc}")
                for (s0, s1) in ((0, 512), (512, T)):
                    cs = [c for c in range(TC8) if c * P < s1]
                    for c in cs:
                        q0 = max(s0, c * P)
                        sl = slice(offs[c] + (q0 - c * P),
                                   offs[c] + (s1 - c * P))
                        nc.tensor.matmul(
                            yps[:, q0:s1], vtm[:, b * TC8 + c, :],
                            expT[:, sl], start=(c == 0), stop=(c == cs[-1]))
                if (b, hc) not in rbs:
                    emit_dps(b, hc)
                rb = rbs.pop((b, hc))
                y16 = att['y16'].tile([P, T], f16, tag="y16",
                                      name=f"y16_{b}_{hc}")
                nc.vector.tensor_mul(y16[:], yps[:], rb[:])
                kp, sl8 = hc // 2, hc % 2
                yh_v = y8h[:, kp, b * TC8:(b + 1) * TC8, sl8, :]
                yl_v = y8l[:, kp, b * TC8:(b + 1) * TC8, sl8, :]
                nc.scalar.copy(yh_v, y16[:])
                with nc.allow_low_precision(
                        reason="fp8 hi/lo split; recon err ~0.1%, tol 2e-2"):
                    nc.gpsimd.tensor_sub(yl_v, y16[:], yh_v)

            def emit_proj(m):
                """out[tokens m*128:(m+1)*128, :] = y.T @ wproj (partial),
                3-term compensated fp8 DoubleRow."""
                psA = att['psA']
                ob = ob_pool.tile([P, NE], f16, tag="ob", name=f"ob{m}")
                for n in range(NE // 512):
                    opsum = psA.tile([P, 512], f32, tag="acc", bufs=6,
                                     name=f"ops{m}_{n}")
                    for kp in range(2):
                        for t, (yt, wt) in enumerate(
                                ((y8h, wp8h), (y8l, wp8h), (y8h, wp8l))):
                            nc.tensor.matmul(
                                opsum[:], yt[:, kp, m], wt[:, kp, n],
                                start=(kp == 0 and t == 0),
                                stop=(kp == 1 and t == 2), perf_mode=DR)
                    if 'tail_alt' in FLAGS and m == NT // P - 1:
                        if n % 2:
                            nc.vector.tensor_copy(
                                ob[:, n * 512:(n + 1) * 512], opsum[:])
                        else:
                            nc.scalar.copy(
                                ob[:, n * 512:(n + 1) * 512], opsum[:])
                    else:
                        nc.any.tensor_copy(
                            ob[:, n * 512:(n + 1) * 512], opsum[:])
                    if 'tail_split' in FLAGS and m == NT // P - 1:
                        c0, c1 = n * 512, (n + 1) * 512
                        nc.sync.dma_start(
                            out_d[m * P:(m + 1) * P, c0:c1], ob[:, c0:c1])
                    elif 'quarter_dma' in FLAGS:
                        if n % 2 == 1:
                            c0, c1 = (n - 1) * 512, (n + 1) * 512
                            nc.sync.dma_start(
                                out_d[m * P:(m + 1) * P, c0:c1], ob[:, c0:c1])
                    elif n == 3:
                        nc.sync.dma_start(
                            out_d[m * P:(m + 1) * P, 0:2048], ob[:, 0:2048])
                    elif n == 7:
                        nc.sync.dma_start(
                            out_d[m * P:(m + 1) * P, 2048:NE], ob[:, 2048:NE])

            # ============ phase 1+2: qkv projection + rope, per batch ========
            with ExitStack() as sA:
                qkv_pool = sA.enter_context(tc.tile_pool(name="qkv", bufs=1))
                qkv = qkv_pool.tile([P, MC, NT], f16)
                wq_pool = sA.enter_context(tc.tile_pool(name="wq", bufs=1))
                wqh = wq_pool.tile([P, KP, MC, 2, P], f8)
                wql = wq_pool.tile([P, KP, MC, 2, P], f8)
                xs_pool = sA.enter_context(tc.tile_pool(name="xs", bufs=4))
                rp = sA.enter_context(tc.tile_pool(name="rope", bufs=2))

                def rope_span(b, tok, w):
                    h = HS // 2
                    ccb, ssb = cc[:, tok], ss[:, tok]
                    # half-spans get their own tags: mixed tile sizes inside
                    # one rotation tag alias SBUF and corrupt data on HW
                    sfx = "" if w == T else "H"
                    for hc in range(QPK + 1):
                        src_ = qkv[:, hc, tok]
                        rot = rp.tile([P, w], f16, tag="rot" + sfx,
                                      name=f"rot{b}_{hc}_{tok.start}")
                        nc.sync.dma_start(rot[0:h, :], src_[h:P, :])
                        nc.sync.dma_start(rot[h:P, :], src_[0:h, :])
                        t1 = rp.tile([P, w], f16, tag="t1" + sfx,
                                     name=f"t1_{b}_{hc}_{tok.start}")
                        t2 = rp.tile([P, w], f16, tag="t2" + sfx,
                                     name=f"t2_{b}_{hc}_{tok.start}")
                        nc.vector.tensor_mul(t1[:], src_, ccb)
                        nc.vector.tensor_mul(t2[:], rot[:], ssb)
                        dst = (q16[:, hc, tok] if hc < QPK
                               else k16[:, tok])
                        with nc.allow_low_precision(
                                reason="fp16 rope; |q|,|k|~1, tol 2e-2"):
                            nc.vector.tensor_add(dst, t1[:], t2[:])

                def rope_batch(b):
                    rope_span(b, slice(b * T, (b + 1) * T), T)

                def vt_batch(b, pool, tag, bufs, shape, cs=None):
                    for c in (range(TC8) if cs is None else cs):
                        # PE transpose (avoids XBAR DMA-transpose, which
                        # races concurrent DMA copies on this stack)
                        vt_ps = pool.tile(shape, f16, tag=tag, bufs=bufs,
                                          name=f"vt{b}_{c}")
                        nc.tensor.transpose(
                            vt_ps[:, 0:P],
                            qkv[:, QPK + 1, b * T + c * P: b * T + (c + 1) * P],
                            ident16[:])
                        nc.any.tensor_copy(
                            vtm[:, b * TC8 + c, :], vt_ps[:, 0:P])

                def ident_dma():
                    nc.sync.dma_start(ident16[:], ident16_d[:])

                def consts_dma():
                    nc.sync.dma_start(cc[:], cc_d[:])
                    nc.sync.dma_start(ss[:], ss_d[:])
                    nc.sync.dma_start(maskT[:], mask_d[:])
                    nc.sync.dma_start(ones2d[:], ones2d_d[:])

                # resident x8 hi/lo tile sets shared by both batches
                def load_xt(n, kp, which):
                    src = x8h_d if which == 'h' else x8l_d
                    xt = xs_pool.tile([P, 2, 512], f8, tag="xt" + which,
                                      bufs=16, name=f"xt{which}{n}_{kp}")
                    nc.sync.dma_start(xt[:], src[n, kp])
                    return xt

                def qkv_mms(psums, ms, kp, xh, xl, first, last):
                    """3-term compensated fp8 DR matmuls for one kp pair."""
                    for m in ms:
                        for t, (wt, xt) in enumerate(
                                ((wqh, xh), (wql, xh), (wqh, xl))):
                            nc.tensor.matmul(
                                psums[m][:], wt[:, kp, m], xt[:],
                                start=(first and t == 0),
                                stop=(last and t == 2), perf_mode=DR)

                # ---- batch 0: flat 6-psum sweeps, own PSUM pool ----
                with ExitStack() as sP0:
                    ps1a = sP0.enter_context(
                        tc.tile_pool(name="ps1a", bufs=6, space="PSUM"))
                    for n in (0, 1):
                        psums = [ps1a.tile([P, 512], f32, tag="ps1",
                                           name=f"ps1_{n}_{m_}")
                                 for m_ in range(MC)]
                        for kp in range(KP):
                            if n == 0:
                                nc.sync.dma_start(wqh[:, kp], wqh_d[kp])
                            xh = load_xt(n, kp, 'h')
                            if n == 0:
                                nc.sync.dma_start(wql[:, kp], wql_d[kp])
                            xl = load_xt(n, kp, 'l')
                            qkv_mms(psums, range(MC), kp, xh, xl,
                                    kp == 0, kp == KP - 1)
                        for m in range(MC):
                            if 'altcopy' in FLAGS and m % 2 == 0:
                                nc.scalar.copy(
                                    qkv[:, m, n * 512:(n + 1) * 512],
                                    psums[m][:])
                            else:
                                nc.vector.tensor_copy(
                                    qkv[:, m, n * 512:(n + 1) * 512],
                                    psums[m][:])
                        if n == 0:
                            # MUST precede the first vt transpose: a
                            # reader emitted before its producer DMA
                            # gets no dependency and reads uninitialized
                            # SBUF (ident16 is vt's identity operand)
                            ident_dma()
                        if 'vt_split' in FLAGS:
                            vt_batch(0, ps1a, "vt", 2, [P, P],
                                     cs=range(n * 4, n * 4 + 4))
                    consts_dma()
                    rope_batch(0)
                    if 'vt_split' not in FLAGS:
                        vt_batch(0, ps1a, "vt", 2, [P, P])

                # attention pools come alive before batch 1 so batch-0
                # scores/exp interleave into batch-1's qkv stream
                make_att_pools()

                # ---- batch 1: three 2-psum m-groups per n-chunk ----
                groups = ((0, 1), (2, 3), (4, 5))
                with ExitStack() as sP1:
                    ps1b = sP1.enter_context(
                        tc.tile_pool(name="ps1b", bufs=2, space="PSUM"))
                    for n in (2, 3):
                        xts = {}
                        for g, ms in enumerate(groups):
                            psums = {m_: ps1b.tile(
                                [P, 512], f32, tag="ps1",
                                name=f"ps1_{n}_{g}_{m_}") for m_ in ms}
                            for kp in range(KP):
                                if g == 0:
                                    xts[kp] = (load_xt(n, kp, 'h'),
                                               load_xt(n, kp, 'l'))
                                qkv_mms(psums, ms, kp, *xts[kp],
                                        kp == 0, kp == KP - 1)
                            for m in ms:
                                nc.any.tensor_copy(
                                    qkv[:, m, n * 512:(n + 1) * 512],
                                    psums[m][:])
                            slot = (n - 2) * 3 + g
                            if slot < QPK:
                                emit_scores(0, slot)
                    rope_batch(1)
                    vt_batch(1, att['psA'], "acc", 6, [P, 1024])

            # yps PSUM pool only comes alive after ps1b frees its banks
            att['psY'] = sR.enter_context(
                tc.tile_pool(name="psY", bufs=1, space="PSUM"))
            y8h, y8l, wp8h, wp8l = alloc_yw_pools()
            ob_pool = sL.enter_context(tc.tile_pool(name="ob", bufs=2))
            for kp in range(2):
                nc.sync.dma_start(wp8h[:, kp], wph_d[kp])
                nc.sync.dma_start(wp8l[:, kp], wpl_d[kp])

            # ===== batch 0 attention (pv) / batch 1 scores interleave =====
            for i in range(QPK):
                emit_pv(0, i)
                emit_scores(1, i)

            # ===== batch 1 attention interleaved with batch-0 proj: proj
            # matmuls keep PE busy while ACT runs exp for the next head =====
            plan = [('pt', 0), ('pj', 0), ('pt', 1), ('pv', 0),
                    ('pj', 1), ('pt', 2), ('pj', 2), ('pv', 1),
                    ('pj', 3), ('pt', 3), ('pj', 4), ('pv', 2),
                    ('pj', 5), ('pj', 6), ('pv', 3), ('pj', 7)]
            for op, i in plan:
                if op == 'pt':
                    if 'late_part' in FLAGS:
                        emit_partial(1, i)
                elif op == 'pv':
                    emit_pv(1, i)
                else:
                    emit_proj(i)
            for m in range(8, NT // P):
                emit_proj(m)
        finally:
            sR.close()
            sL.close()

    if split_waits:
        _split_waits(nc, mybir)
    return nc


def _q8(v):
    import ml_dtypes
    return np.ascontiguousarray(v).astype(ml_dtypes.float8_e4m3)


def _split8(v):
    """2-level e4m3 decomposition: v ~= hi + lo."""
    hi = _q8(v)
    lo = _q8(v - hi.astype(np.float32))
    return hi, lo


def _host_prep(x, cos, sin, W_attn, W_proj):
    xT = np.ascontiguousarray(x.reshape(NT, NE).T)          # [NE, NT] f32
    # x8[n, kp, p, i, c] = e4m3(xT[(2kp+i)*P + p, n*512 + c])
    xr = xT.reshape(KP, 2, P, NNC, 512).transpose(3, 0, 2, 1, 4)
    x8h, x8l = _split8(xr)
    cosT = np.tile(cos.T, (1, B)) / WSCALE
    sinT = np.tile(sin.T, (1, B)) / WSCALE
    cc = np.ascontiguousarray(
        np.concatenate([cosT, cosT], axis=0), dtype=np.float16)
    ss = np.ascontiguousarray(
        np.concatenate([-sinT, sinT], axis=0), dtype=np.float16)
    # scoresT layout [kv, q]: zero strictly-lower (kv > q) entries post-exp
    maskT = np.triu(np.ones((P, P), dtype=np.float16))
    common = {"x8h": x8h, "x8l": x8l, "cc": cc, "ss": ss, "maskT": maskT,
              "ident16": np.eye(P, dtype=np.float16),
              "ones2d": np.ones((P, P), dtype=np.float16)}
    in_maps = []
    for g in range(NCORES):
        m = dict(common)
        wq = W_attn[g * GW:(g + 1) * GW, :].T * WSCALE      # [NE, GW] f32
        # wq8[kp, p, m, i, j] = e4m3(32*wq[(2kp+i)*P+p, m*128+j])
        wqr = wq.reshape(KP, 2, P, MC, P).transpose(0, 2, 3, 1, 4)
        m["wqh"], m["wql"] = _split8(wqr)
        wp = W_proj[:, g * GQ:(g + 1) * GQ].T * WSCALE      # [GQ, NE] f32
        # wp8[kp, p, nn, i, c] = e4m3(32*wp[(2kp+i)*P+p, nn*512+c])
        wpr = wp.reshape(2, 2, P, NE // 512, 512).transpose(0, 2, 3, 1, 4)
        m["wph"], m["wpl"] = _split8(wpr)
        in_maps.append(m)
    return in_maps


LAST_EXEC_NS = None


def kernel(x, cos, sin, W_attn, W_proj, max_seq_length):
    global LAST_EXEC_NS
    from concourse.bass_utils import run_bass_kernel_spmd

    x = np.asarray(x, dtype=np.float32)
    cos = np.asarray(cos, dtype=np.float32)
    sin = np.asarray(sin, dtype=np.float32)
    W_attn = np.asarray(W_attn, dtype=np.float32)
    W_proj = np.asarray(W_proj, dtype=np.float32)

    if "nc" not in _CACHE:
        _CACHE["nc"] = _build_nc()
    nc = _CACHE["nc"]

    in_maps = _host_prep(x, cos, sin, W_attn, W_proj)
    res = run_bass_kernel_spmd(nc, in_maps, core_ids=list(range(NCORES)))
    LAST_EXEC_NS = res.exec_time_ns

    acc = res.results[0]["out"].astype(np.float32)
    for g in range(1, NCORES):
        acc = acc + res.results[g]["out"].astype(np.float32)
    return acc.reshape(B, T, NE) * (1.0 / (WSCALE * WSCALE))


# revision 10
# speedup vs baseline: 1.2022x; 1.0541x over previous
"""Trainium2 Bass kernel for CausalSelfAttention (GQA, RoPE, prefill).

Tensor-parallel over the 8 query groups: core g owns query heads
[4g, 4g+4) and kv head g.  Each core computes a partial output
(full-shape, fp16) that the host sums in fp32.

The two dense projections (qkv: x@W_attn slice, proj: y@W_proj slice)
run as 3-term error-compensated fp8 DoubleRow matmuls:
    x @ W  ~=  x_hi@W_hi + x_lo@W_hi + x_hi@W_lo
with x_hi = e4m3(x), x_lo = e4m3(x - x_hi) (same for W, pre-scaled by
32 on host so W's ~N(0, 1/4096) entries stay in e4m3 normal range).
DoubleRow packs two (128-contraction-plane, term) pairs per PE
instruction at 0.5 cycles per output column, so each term costs 1/4 of
an fp16 matmul and the compensated product runs at 0.75x fp16 time with
~0.2% error (vs ~4% for naive fp8).  Scale bookkeeping: q,k are
descaled by folding 1/32 into the host cos/sin tables; v stays 32x and
W_proj adds another 32x, so the host divides the summed output by 1024.

Attention stays fp16: scores KV-MAJOR (scoresT = kT.T @ qT, 6-deep PSUM
rotation), exp on ACT straight into the PV rhs layout, causal-diagonal
mask on GpSimd, softmax denominator via DVE block pre-reduce + one
ones-matmul (cross-partition reduce + broadcast) + DVE reciprocal.
emit_pv additionally splits y into fp8 hi/lo for the proj stage
(DVE mul, ACT quantize-copy, GpSimd residual-subtract).

Schedule skeleton (every engine stream is in-order, so EMISSION ORDER
is the schedule): batch-0 qkv runs flat 6-PSUM kp-sweeps; batch 1 runs
three 2-PSUM m-groups per n-chunk with batch-0 attention interleaved;
batch-0 proj chunks interleave into batch-1's attention as PE filler.
"""

import os
import numpy as np

FLAGS = set(os.environ.get(
    'KFLAGS',
    'half_dma,tail_split,tail_alt,altcopy,late_part,vt_split').split(','))

B, T, NE, NH, NQG, HS = 2, 1024, 4096, 32, 8, 128
QPK = NH // NQG          # 4 query heads per kv group
NT = B * T               # 2048 tokens
GW = (QPK + 2) * HS      # 768 qkv rows per group
GQ = QPK * HS            # 512 q cols per group
P = 128
NCORES = 8
KC = NE // P             # 32 contraction chunks for qkv proj
KP = KC // 2             # 16 DoubleRow plane-pairs
MC = GW // P             # 6 qkv feature chunks
TC8 = T // P             # 8 token chunks per batch
NNC = NT // 512          # 4 token n-chunks
SCALE = 1.0 / float(np.sqrt(HS))
WSCALE = 32.0            # host pre-scale on W_attn / W_proj before e4m3

_CACHE = {}


def _split_waits(nc, mybir, max_waits=1):
    """walrus in this container rejects >1 sync-wait per instruction;
    hoist extras onto single-wait NoOps just before (equivalent since
    semaphores are monotonic and a sequencer executes in order)."""
    for fn in nc.m.functions:
        for blk in fn.blocks:
            new_list, changed = [], False
            for inst in blk.instructions:
                si = getattr(inst, "sync_info", None)
                if si is not None and len(si.on_wait) > max_waits:
                    waits = list(si.on_wait)
                    for i, w in enumerate(waits[:-max_waits]):
                        nop = mybir.InstNoOp(
                            name=f"{inst.name}-wsplit-{i}", ins=[], outs=[],
                            engine=inst.engine)
                        nop.sync_info = mybir.SyncInfo(on_wait=[w], on_update=[])
                        new_list.append(nop)
                    inst.sync_info = mybir.SyncInfo(
                        on_wait=waits[-max_waits:], on_update=list(si.on_update))
                    changed = True
                new_list.append(inst)
            if changed:
                blk.instructions = new_list


def _build_nc(reps=1, split_waits=True):
    import concourse.bass as bass
    import concourse.mybir as mybir
    import concourse.tile as tile
    from contextlib import ExitStack

    f32 = mybir.dt.float32
    f16 = mybir.dt.float16
    f8 = mybir.dt.float8e4
    DR = mybir.MatmulPerfMode.DoubleRow

    nc = bass.Bass()
    # fp8 pair-packed inputs, hi/lo fused per DMA (see _host_prep)
    x8_d = nc.dram_tensor("x8", [NNC, KP, P, 2, 2, 512], f8,
                          kind="ExternalInput")
    wq8_d = nc.dram_tensor("wq8", [KP, P, 2, MC, 2, P], f8,
                           kind="ExternalInput")
    wp8_d = nc.dram_tensor("wp8", [2, P, 2, NE // 512, 2, 512], f8,
                           kind="ExternalInput")
    css_d = nc.dram_tensor("css", [P, 2, NT], f16, kind="ExternalInput")
    misc_d = nc.dram_tensor("misc", [P, 3, P], f16, kind="ExternalInput")
    out_d = nc.dram_tensor("out", [NT, NE], f16, kind="ExternalOutput")

    # column offset of kv-chunk c's block inside the expT tile
    offs, acc = [], 0
    for c in range(TC8):
        offs.append(acc)
        acc += (TC8 - c) * P

    with tile.TileContext(nc) as tc:
      for _rep in range(reps):
        sL = ExitStack()   # left-side long-lived pools (y8, wp8, ob)
        sR = ExitStack()   # right-side pools (qk16, attention-era)
        try:
            # const: left
            const = sL.enter_context(tc.tile_pool(name="const", bufs=1))
            css = const.tile([P, 2, NT], f16)
            cc, ss = css[:, 0], css[:, 1]
            misc = const.tile([P, 3, P], f16)
            maskT, ones2d, ident16 = misc[:, 0], misc[:, 1], misc[:, 2]

            def alloc_yw_pools():
                y_pool = sL.enter_context(tc.tile_pool(name="y", bufs=1))
                # (p, kp-pair, token-chunk, slot, col) — proj lhsT slices
                y8h = y_pool.tile([P, 2, NT // P, 2, P], f8)
                y8l = y_pool.tile([P, 2, NT // P, 2, P], f8)
                wp_pool = sL.enter_context(tc.tile_pool(name="wp", bufs=1))
                wp8 = wp_pool.tile([P, 2, 2, NE // 512, 2, 512], f8)
                return y8h, y8l, wp8

            # qk16 on the right: lives through attention
            qk16 = sR.enter_context(tc.tile_pool(name="qk16", bufs=1, side="right"))
            q16 = qk16.tile([P, QPK, NT], f16)
            k16 = qk16.tile([P, NT], f16)
            vtm = qk16.tile([P, B * TC8, P], f16)

            att = {}

            def make_att_pools():
                att['expT'] = sR.enter_context(
                    tc.tile_pool(name="expT", bufs=5, side="right"))
                att['part'] = sR.enter_context(
                    tc.tile_pool(name="part", bufs=2, side="right"))
                att['rb'] = sR.enter_context(
                    tc.tile_pool(name="rb", bufs=2, side="right"))
                att['y16'] = sR.enter_context(
                    tc.tile_pool(name="y16", bufs=2, side="right"))
                att['psA'] = sR.enter_context(
                    tc.tile_pool(name="psA", bufs=1, space="PSUM"))

            expTs = {}
            parts = {}
            rbs = {}

            def emit_partial(b, hc):
                """DVE pre-reduce of the kv-chunk blocks into partial[128,T]
                (softmax denominator before the cross-partition reduce).
                Emitted only once exp data is near-ready so the in-order DVE
                queue is not head-of-line blocked."""
                expT = expTs[(b, hc)]
                part = att['part'].tile([P, T], f16, tag="part",
                                        name=f"part{b}_{hc}")
                parts[(b, hc)] = part
                nc.vector.tensor_copy(part[:], expT[:, offs[0]:offs[0] + T])
                with nc.allow_low_precision(
                        reason="fp16 partial rowsums; d<=~3e3, tol 2e-2"):
                    for c in range(1, TC8):
                        w = T - c * P
                        nc.vector.tensor_add(
                            part[:, c * P:T], part[:, c * P:T],
                            expT[:, offs[c]:offs[c] + w])

            def emit_dps(b, hc):
                """softmax denominator: ones-matmul does the cross-partition
                reduce AND the 128-way broadcast; DVE reciprocal to SBUF."""
                psA = att['psA']
                part = parts.pop((b, hc))
                rb = att['rb'].tile([P, T], f16, tag="rb", name=f"rb{b}_{hc}")
                rbs[(b, hc)] = rb
                for (q0, q1) in ((0, 512), (512, T)):
                    dps = psA.tile([P, 512], f32, tag="acc", bufs=6,
                                   name=f"dps{b}_{hc}_{q0}")
                    nc.tensor.matmul(dps[:], ones2d, part[:, q0:q1],
                                     start=True, stop=True)
                    with nc.allow_low_precision(
                            reason="fp16 1/d; d in [1,~3e3], tol 2e-2"):
                        nc.vector.reciprocal(rb[:, q0:q1], dps[:])

            def emit_scores(b, hc):
                """scoresT = kT.T @ qT per kv chunk, exp on ACT, causal mask
                on GpSimd."""
                psA = att['psA']
                tok = slice(b * T, (b + 1) * T)
                qT_i = q16[:, hc, tok]
                expT = att['expT'].tile([P, acc], f16, tag="expT",
                                        name=f"expT{b}_{hc}")
                expTs[(b, hc)] = expT
                for c in range(TC8):
                    kT_c = k16[:, b * T + c * P: b * T + (c + 1) * P]
                    spans = [(c * P, 512)] if c < 4 else []
                    spans += [(max(512, c * P), T)]
                    for si, (q0, q1) in enumerate(spans):
                        sps = psA.tile([P, 512], f32, tag="acc", bufs=6,
                                       name=f"sps{b}_{hc}_{c}_{q0}")
                        w = q1 - q0
                        nc.tensor.matmul(sps[:, :w], kT_c,
                                         qT_i[:, q0:q1],
                                         start=True, stop=True)
                        eo = offs[c] + (q0 - c * P)
                        nc.scalar.activation(
                            expT[:, eo:eo + w], sps[:, :w],
                            mybir.ActivationFunctionType.Exp, scale=SCALE)
                    # zero the invalid (kv > q) half of the diagonal block
                    nc.gpsimd.tensor_mul(
                        expT[:, offs[c]:offs[c] + P],
                        expT[:, offs[c]:offs[c] + P], maskT)
                if not (b == 1 and 'late_part' in FLAGS):
                    emit_partial(b, hc)

            def emit_pv(b, hc):
                """y = probs @ v (unnormalized), denominator reduce+broadcast
                via ones-matmul, DVE reciprocal, normalizing multiply into a
                fp16 scratch, then fp8 hi/lo split for the proj stage."""
                psA = att['psA']
                if (b, hc) not in parts and (b, hc) not in rbs:
                    emit_partial(b, hc)
                expT = expTs.pop((b, hc))
                yps = att['psY'].tile([P, T], f32, tag="yps", bufs=1,
                                      name=f"yps{b}_{hc}")
                for (s0, s1) in ((0, 512), (512, T)):
                    cs = [c for c in range(TC8) if c * P < s1]
                    for c in cs:
                        q0 = max(s0, c * P)
                        sl = slice(offs[c] + (q0 - c * P),
                                   offs[c] + (s1 - c * P))
                        nc.tensor.matmul(
                            yps[:, q0:s1], vtm[:, b * TC8 + c, :],
                            expT[:, sl], start=(c == 0), stop=(c == cs[-1]))
                if (b, hc) not in rbs:
                    emit_dps(b, hc)
                rb = rbs.pop((b, hc))
                y16 = att['y16'].tile([P, T], f16, tag="y16",
                                      name=f"y16_{b}_{hc}")
                nc.vector.tensor_mul(y16[:], yps[:], rb[:])
                kp, sl8 = hc // 2, hc % 2
                yh_v = y8h[:, kp, b * TC8:(b + 1) * TC8, sl8, :]
                yl_v = y8l[:, kp, b * TC8:(b + 1) * TC8, sl8, :]
                nc.scalar.copy(yh_v, y16[:])
                with nc.allow_low_precision(
                        reason="fp8 hi/lo split; recon err ~0.1%, tol 2e-2"):
                    nc.gpsimd.tensor_sub(yl_v, y16[:], yh_v)

            def emit_proj(m):
                """out[tokens m*128:(m+1)*128, :] = y.T @ wproj (partial),
                3-term compensated fp8 DoubleRow."""
                psA = att['psA']
                ob = ob_pool.tile([P, NE], f16, tag="ob", name=f"ob{m}")
                for n in range(NE // 512):
                    opsum = psA.tile([P, 512], f32, tag="acc", bufs=6,
                                     name=f"ops{m}_{n}")
                    for kp in range(2):
                        for t, (yt, wv) in enumerate(
                                ((y8h, 0), (y8l, 0), (y8h, 1))):
                            nc.tensor.matmul(
                                opsum[:], yt[:, kp, m], wp8[:, kp, wv, n],
                                start=(kp == 0 and t == 0),
                                stop=(kp == 1 and t == 2), perf_mode=DR)
                    if 'tail_alt' in FLAGS and m == NT // P - 1:
                        if n % 2:
                            nc.vector.tensor_copy(
                                ob[:, n * 512:(n + 1) * 512], opsum[:])
                        else:
                            nc.scalar.copy(
                                ob[:, n * 512:(n + 1) * 512], opsum[:])
                    else:
                        nc.any.tensor_copy(
                            ob[:, n * 512:(n + 1) * 512], opsum[:])
                    if 'tail_split' in FLAGS and m == NT // P - 1:
                        c0, c1 = n * 512, (n + 1) * 512
                        nc.sync.dma_start(
                            out_d[m * P:(m + 1) * P, c0:c1], ob[:, c0:c1])
                    elif 'quarter_dma' in FLAGS:
                        if n % 2 == 1:
                            c0, c1 = (n - 1) * 512, (n + 1) * 512
                            nc.sync.dma_start(
                                out_d[m * P:(m + 1) * P, c0:c1], ob[:, c0:c1])
                    elif 'half_dma' in FLAGS and n % 4 == 3:
                        c0, c1 = (n - 3) * 512, (n + 1) * 512
                        nc.sync.dma_start(
                            out_d[m * P:(m + 1) * P, c0:c1], ob[:, c0:c1])
                    elif n == 3:
                        nc.sync.dma_start(
                            out_d[m * P:(m + 1) * P, 0:2048], ob[:, 0:2048])
                    elif n == 7:
                        nc.sync.dma_start(
                            out_d[m * P:(m + 1) * P, 2048:NE], ob[:, 2048:NE])

            # ============ phase 1+2: qkv projection + rope, per batch ========
            with ExitStack() as sA:
                qkv_pool = sA.enter_context(tc.tile_pool(name="qkv", bufs=1))
                qkv = qkv_pool.tile([P, MC, NT], f16)
                wq_pool = sA.enter_context(tc.tile_pool(name="wq", bufs=1))
                wq8 = wq_pool.tile([P, KP, 2, MC, 2, P], f8)
                xs_pool = sA.enter_context(tc.tile_pool(name="xs", bufs=4))
                rp = sA.enter_context(tc.tile_pool(name="rope", bufs=2))

                def rope_span(b, tok, w):
                    h = HS // 2
                    ccb, ssb = cc[:, tok], ss[:, tok]
                    # half-spans get their own tags: mixed tile sizes inside
                    # one rotation tag alias SBUF and corrupt data on HW
                    sfx = "" if w == T else "H"
                    for hc in [QPK] + list(range(QPK)):
                        src_ = qkv[:, hc, tok]
                        rot = rp.tile([P, w], f16, tag="rot" + sfx,
                                      name=f"rot{b}_{hc}_{tok.start}")
                        nc.sync.dma_start(rot[0:h, :], src_[h:P, :])
                        nc.sync.dma_start(rot[h:P, :], src_[0:h, :])
                        t1 = rp.tile([P, w], f16, tag="t1" + sfx,
                                     name=f"t1_{b}_{hc}_{tok.start}")
                        t2 = rp.tile([P, w], f16, tag="t2" + sfx,
                                     name=f"t2_{b}_{hc}_{tok.start}")
                        nc.vector.tensor_mul(t1[:], src_, ccb)
                        nc.vector.tensor_mul(t2[:], rot[:], ssb)
                        dst = (q16[:, hc, tok] if hc < QPK
                               else k16[:, tok])
                        with nc.allow_low_precision(
                                reason="fp16 rope; |q|,|k|~1, tol 2e-2"):
                            nc.vector.tensor_add(dst, t1[:], t2[:])

                def rope_batch(b):
                    rope_span(b, slice(b * T, (b + 1) * T), T)

                def vt_batch(b, pool, tag, bufs, shape, cs=None):
                    for c in (range(TC8) if cs is None else cs):
                        # PE transpose (avoids XBAR DMA-transpose, which
                        # races concurrent DMA copies on this stack)
                        vt_ps = pool.tile(shape, f16, tag=tag, bufs=bufs,
                                          name=f"vt{b}_{c}")
                        nc.tensor.transpose(
                            vt_ps[:, 0:P],
                            qkv[:, QPK + 1, b * T + c * P: b * T + (c + 1) * P],
                            ident16)
                        nc.any.tensor_copy(
                            vtm[:, b * TC8 + c, :], vt_ps[:, 0:P])

                def consts_dma():
                    # one fused DMA for cos/sin tables, one for mask/ones/ident
                    nc.sync.dma_start(css[:], css_d[:])
                    nc.sync.dma_start(misc[:], misc_d[:])

                # resident x8 hi/lo tile sets shared by both batches;
                # hi+lo fused into one DMA to halve HWDGE desc-gen slots
                def load_xt(n, kp):
                    xt = xs_pool.tile([P, 2, 2, 512], f8, tag="xt",
                                      bufs=16, name=f"xt{n}_{kp}")
                    nc.sync.dma_start(xt[:], x8_d[n, kp])
                    return xt[:, 0], xt[:, 1]

                def qkv_mms(psums, ms, kp, xh, xl, first, last):
                    """3-term compensated fp8 DR matmuls for one kp pair."""
                    for m in ms:
                        for t, (wv, xt) in enumerate(
                                ((0, xh), (1, xh), (0, xl))):
                            nc.tensor.matmul(
                                psums[m][:], wq8[:, kp, wv, m], xt,
                                start=(first and t == 0),
                                stop=(last and t == 2), perf_mode=DR)

                # ---- batch 0: flat 6-psum sweeps, own PSUM pool ----
                with ExitStack() as sP0:
                    ps1a = sP0.enter_context(
                        tc.tile_pool(name="ps1a", bufs=6, space="PSUM"))
                    for n in (0, 1):
                        psums = [ps1a.tile([P, 512], f32, tag="ps1",
                                           name=f"ps1_{n}_{m_}")
                                 for m_ in range(MC)]
                        for kp in range(KP):
                            if n == 0:
                                nc.sync.dma_start(wq8[:, kp], wq8_d[kp])
                            xh, xl = load_xt(n, kp)
                            qkv_mms(psums, range(MC), kp, xh, xl,
                                    kp == 0, kp == KP - 1)
                        for m in range(MC):
                            if 'altcopy' in FLAGS and m % 2 == 0:
                                nc.scalar.copy(
                                    qkv[:, m, n * 512:(n + 1) * 512],
                                    psums[m][:])
                            else:
                                nc.vector.tensor_copy(
                                    qkv[:, m, n * 512:(n + 1) * 512],
                                    psums[m][:])
                        if n == 0:
                            # MUST precede the first vt transpose: a
                            # reader emitted before its producer DMA
                            # gets no dependency and reads uninitialized
                            # SBUF (ident16 is vt's identity operand)
                            consts_dma()
                        if 'vt_split' in FLAGS:
                            vt_batch(0, ps1a, "vt", 2, [P, P],
                                     cs=range(n * 4, n * 4 + 4))
                    rope_batch(0)
                    if 'vt_split' not in FLAGS:
                        vt_batch(0, ps1a, "vt", 2, [P, P])

                # attention pools come alive before batch 1 so batch-0
                # scores/exp interleave into batch-1's qkv stream
                make_att_pools()

                # ---- batch 1: three 2-psum m-groups per n-chunk ----
                groups = ((0, 1), (2, 3), (4, 5))
                with ExitStack() as sP1:
                    ps1b = sP1.enter_context(
                        tc.tile_pool(name="ps1b", bufs=2, space="PSUM"))
                    for n in (2, 3):
                        xts = {}
                        for g, ms in enumerate(groups):
                            psums = {m_: ps1b.tile(
                                [P, 512], f32, tag="ps1",
                                name=f"ps1_{n}_{g}_{m_}") for m_ in ms}
                            for kp in range(KP):
                                if g == 0:
                                    xts[kp] = load_xt(n, kp)
                                qkv_mms(psums, ms, kp, *xts[kp],
                                        kp == 0, kp == KP - 1)
                            for m in ms:
                                nc.any.tensor_copy(
                                    qkv[:, m, n * 512:(n + 1) * 512],
                                    psums[m][:])
                            slot = (n - 2) * 3 + g
                            if slot < QPK:
                                emit_scores(0, slot)
                    rope_batch(1)
                    vt_batch(1, att['psA'], "acc", 6, [P, 1024])

            # yps PSUM pool only comes alive after ps1b frees its banks
            att['psY'] = sR.enter_context(
                tc.tile_pool(name="psY", bufs=1, space="PSUM"))
            y8h, y8l, wp8 = alloc_yw_pools()
            ob_pool = sL.enter_context(tc.tile_pool(name="ob", bufs=2))
            for kp in range(2):
                nc.sync.dma_start(wp8[:, kp], wp8_d[kp])

            # ===== batch 0 attention (pv) / batch 1 scores interleave =====
            for i in range(QPK):
                emit_pv(0, i)
                emit_scores(1, i)

            # ===== batch 1 attention interleaved with batch-0 proj: proj
            # matmuls keep PE busy while ACT runs exp for the next head =====
            plan = [('pt', 0), ('pj', 0), ('pt', 1), ('pv', 0),
                    ('pj', 1), ('pt', 2), ('pj', 2), ('pv', 1),
                    ('pj', 3), ('pt', 3), ('pj', 4), ('pv', 2),
                    ('pj', 5), ('pj', 6), ('pv', 3), ('pj', 7)]
            for op, i in plan:
                if op == 'pt':
                    if 'late_part' in FLAGS:
                        emit_partial(1, i)
                elif op == 'pv':
                    emit_pv(1, i)
                else:
                    emit_proj(i)
            for m in range(8, NT // P):
                emit_proj(m)
        finally:
            sR.close()
            sL.close()

    if split_waits:
        _split_waits(nc, mybir)
    return nc


def _q8(v):
    import ml_dtypes
    return np.ascontiguousarray(v).astype(ml_dtypes.float8_e4m3)


def _split8(v):
    """2-level e4m3 decomposition: v ~= hi + lo."""
    hi = _q8(v)
    lo = _q8(v - hi.astype(np.float32))
    return hi, lo


def _host_prep(x, cos, sin, W_attn, W_proj):
    xT = np.ascontiguousarray(x.reshape(NT, NE).T)          # [NE, NT] f32
    # x8[n, kp, p, which, i, c] = e4m3{,resid}(xT[(2kp+i)*P + p, n*512 + c])
    xr = xT.reshape(KP, 2, P, NNC, 512).transpose(3, 0, 2, 1, 4)
    x8h, x8l = _split8(xr)
    x8 = np.ascontiguousarray(np.stack([x8h, x8l], axis=3))
    cosT = np.tile(cos.T, (1, B)) / WSCALE
    sinT = np.tile(sin.T, (1, B)) / WSCALE
    cc = np.concatenate([cosT, cosT], axis=0)
    ss = np.concatenate([-sinT, sinT], axis=0)
    css = np.ascontiguousarray(
        np.stack([cc, ss], axis=1), dtype=np.float16)
    # scoresT layout [kv, q]: zero strictly-lower (kv > q) entries post-exp
    maskT = np.triu(np.ones((P, P), dtype=np.float32))
    misc = np.ascontiguousarray(np.stack(
        [maskT, np.ones((P, P), dtype=np.float32), np.eye(P)],
        axis=1), dtype=np.float16)
    common = {"x8": x8, "css": css, "misc": misc}
    in_maps = []
    for g in range(NCORES):
        m = dict(common)
        wq = W_attn[g * GW:(g + 1) * GW, :].T * WSCALE      # [NE, GW] f32
        # wq8[kp, p, which, m, i, j] = e4m3{,resid}(32*wq[(2kp+i)*P+p, m*128+j])
        wqr = wq.reshape(KP, 2, P, MC, P).transpose(0, 2, 3, 1, 4)
        m["wq8"] = np.ascontiguousarray(np.stack(_split8(wqr), axis=2))
        wp = W_proj[:, g * GQ:(g + 1) * GQ].T * WSCALE      # [GQ, NE] f32
        # wp8[kp, p, which, nn, i, c] = e4m3{,resid}(32*wp[(2kp+i)*P+p, nn*512+c])
        wpr = wp.reshape(2, 2, P, NE // 512, 512).transpose(0, 2, 3, 1, 4)
        m["wp8"] = np.ascontiguousarray(np.stack(_split8(wpr), axis=2))
        in_maps.append(m)
    return in_maps


LAST_EXEC_NS = None


def kernel(x, cos, sin, W_attn, W_proj, max_seq_length):
    global LAST_EXEC_NS
    from concourse.bass_utils import run_bass_kernel_spmd

    x = np.asarray(x, dtype=np.float32)
    cos = np.asarray(cos, dtype=np.float32)
    sin = np.asarray(sin, dtype=np.float32)
    W_attn = np.asarray(W_attn, dtype=np.float32)
    W_proj = np.asarray(W_proj, dtype=np.float32)

    if "nc" not in _CACHE:
        _CACHE["nc"] = _build_nc()
    nc = _CACHE["nc"]

    in_maps = _host_prep(x, cos, sin, W_attn, W_proj)
    res = run_bass_kernel_spmd(nc, in_maps, core_ids=list(range(NCORES)))
    LAST_EXEC_NS = res.exec_time_ns

    acc = res.results[0]["out"].astype(np.float32)
    for g in range(1, NCORES):
        acc = acc + res.results[g]["out"].astype(np.float32)
    return acc.reshape(B, T, NE) * (1.0 / (WSCALE * WSCALE))
